# revision 1
# baseline (speedup 1.0000x reference)
"""Self-contained GAT (PyG GATConv, concat=False) Bass/Tile kernel for 8
Trainium2 NeuronCores.  kernel(**inputs) takes the full-graph inputs and
returns the full [N, 32] output.

Strategy (dst-sharded): nodes are packed in id order into groups of <=128
nodes and <=T*128 incoming edges (self-loops included); groups are dealt
contiguously to the 8 cores; every core runs the identical program and all
per-core differences live in the input data (gather indices).  Phase A
computes rows [h | a_src | a_dst] = [x@W | x@W@Acat] into a DRAM scratch
(replicated per core).  Phase B gathers the source rows of each edge tile
with one 128-row indirect DMA per tile, builds the one-hot matrix
S[e,n]=(dst_local==n) with a vector compare, and reduces scatter-softmax +
segment-sum to PE matmuls accumulated in PSUM; softmax max-subtraction is
skipped (alpha is shift-invariant; values are small) and normalization
happens once per node after aggregation.
"""

import math

import numpy as np

import concourse.tile as tile
import concourse.mybir as mb
from concourse import bass, mybir

P = 128
F32 = mybir.dt.float32
I32 = mybir.dt.int32

HEADS = 8
OUT_C = 32
HC = HEADS * OUT_C          # 256
ROW = HC + 2 * HEADS        # 272 f32 per ha row: h | a_src | a_dst
NEG_SLOPE = 0.2
DENOM_EPS = 1e-30


# ----------------------------------------------------------------------------
# Host-side preprocessing: edge grouping + per-core index tensors
# ----------------------------------------------------------------------------

def plan_groups(edge_index: np.ndarray, n_nodes: int, tiles_per_group: int,
                n_cores: int = 8):
    """Pack nodes (in id order) into groups of <=128 nodes and <=T*128 edges
    (self-loops included).  Returns the static geometry + per-core tensors."""
    T = tiles_per_group
    cap = T * P

    src = np.asarray(edge_index[0], dtype=np.int64)
    dst = np.asarray(edge_index[1], dtype=np.int64)
    loops = np.arange(n_nodes, dtype=np.int64)
    src = np.concatenate([src, loops])
    dst = np.concatenate([dst, loops])
    order = np.argsort(dst, kind="stable")
    src_s = src[order].astype(np.int32)
    dst_s = dst[order].astype(np.int32)
    deg = np.bincount(dst_s, minlength=n_nodes).astype(np.int64)
    if deg.max() > cap:
        raise ValueError(f"max degree {deg.max()} exceeds group capacity {cap}")

    # group boundaries over nodes
    starts = [0]
    cur_nodes = 0
    cur_edges = 0
    for n in range(n_nodes):
        d = int(deg[n])
        if cur_nodes == P or cur_edges + d > cap:
            starts.append(n)
            cur_nodes = 0
            cur_edges = 0
        cur_nodes += 1
        cur_edges += d
    starts.append(n_nodes)
    n_groups = len(starts) - 1
    G = math.ceil(n_groups / n_cores)          # group slots per core

    csr = np.zeros(n_nodes + 1, dtype=np.int64)
    np.cumsum(deg, out=csr[1:])

    # per-core tensors
    gidx = np.zeros((n_cores, P, G * T), dtype=np.int32)   # ha row of edge src
    dstl = np.full((n_cores, P, G * T), -1.0, dtype=np.float32)
    nidx = np.zeros((n_cores, P, G), dtype=np.int32)       # node id per (g, p)
    node_of = np.full((n_cores, G, P), -1, dtype=np.int64) # for unshard

    for g_glob in range(n_groups):
        core, g = divmod(g_glob, G) if False else (g_glob // G, g_glob % G)
        n0, n1 = starts[g_glob], starts[g_glob + 1]
        e0, e1 = int(csr[n0]), int(csr[n1])
        ne = e1 - e0
        assert n1 - n0 <= P and ne <= cap
        # edge slot k = p*T + t  (partition-major, matching idx AP ravel order)
        es = src_s[e0:e1]
        ed = (dst_s[e0:e1] - n0).astype(np.float32)
        k = np.arange(ne)
        p_, t_ = k // T, k % T
        gidx[core, p_, g * T + t_] = es
        dstl[core, p_, g * T + t_] = ed
        nn = n1 - n0
        nidx[core, :nn, g] = np.arange(n0, n1, dtype=np.int32)
        node_of[core, g, :nn] = np.arange(n0, n1)

    return dict(G=G, T=T, n_groups=n_groups, gidx=gidx, dstl=dstl, nidx=nidx,
                node_of=node_of)


def host_constants(W, att_src, att_dst, bias):
    """Input-rearrangement constants shipped to every core."""
    W = np.asarray(W, dtype=np.float32)
    acat = np.zeros((2 * P, 2 * HEADS), dtype=np.float32)   # [256, 16]
    for h in range(HEADS):
        acat[h * OUT_C:(h + 1) * OUT_C, h] = np.asarray(att_src)[h]
        acat[h * OUT_C:(h + 1) * OUT_C, HEADS + h] = np.asarray(att_dst)[h]
    # [256,16] -> [128,32] (two column blocks stacked side by side)
    acat2 = np.concatenate([acat[:P], acat[P:]], axis=1).astype(np.float32)
    ident = np.eye(P, dtype=np.float32)
    iota_f = np.tile(np.arange(P, dtype=np.float32), (P, 1))
    bias_rep = np.tile(np.asarray(bias, dtype=np.float32)[None, :], (P, 1))
    return dict(w=W, acat2=acat2, ident=ident, iota_f=iota_f,
                bias_rep=bias_rep)


# ----------------------------------------------------------------------------
# Bass program
# ----------------------------------------------------------------------------

def build_bass(n_nodes: int, G: int, T: int, gather_chunk_tiles: int = 17):
    """Build the (core-id independent) Bass program."""
    nx = math.ceil(n_nodes / P)
    nodes_pad = nx * P

    nc = bass.Bass(trn_type="TRN2", dynamic_dma_scratch_size=65536)

    x_d = nc.dram_tensor("x", [n_nodes, P], F32, kind="ExternalInput")
    w_d = nc.dram_tensor("w", [P, HC], F32, kind="ExternalInput")
    acat_d = nc.dram_tensor("acat2", [P, 4 * HEADS], F32, kind="ExternalInput")
    ident_d = nc.dram_tensor("ident", [P, P], F32, kind="ExternalInput")
    iota_d = nc.dram_tensor("iota_f", [P, P], F32, kind="ExternalInput")
    bias_d = nc.dram_tensor("bias_rep", [P, OUT_C], F32, kind="ExternalInput")
    gidx_d = nc.dram_tensor("gidx", [P, G * T], I32, kind="ExternalInput")
    dstl_d = nc.dram_tensor("dstl", [P, G * T], F32, kind="ExternalInput")
    nidx_d = nc.dram_tensor("nidx", [P, G], I32, kind="ExternalInput")
    out_d = nc.dram_tensor("out", [G * P, OUT_C], F32, kind="ExternalOutput")

    ha_d = nc.dram_tensor("ha", [nodes_pad, ROW], F32)     # internal scratch

    # ---------------- Phase A: ha = [x@W | x@W@Acat] ----------------
    with tile.TileContext(nc) as tc:
        with (
            tc.tile_pool(name="aconst", bufs=1) as cpool,
            tc.tile_pool(name="asb", bufs=3) as spool,
            tc.tile_pool(name="aps0", bufs=1, space="PSUM") as ppool0,
            tc.tile_pool(name="aps", bufs=3, space="PSUM") as ppool,
        ):
            w_sb = cpool.tile([P, HC], F32)
            nc.sync.dma_start(out=w_sb[:], in_=w_d[:, :])
            acat_sb = cpool.tile([P, 4 * HEADS], F32)
            nc.sync.dma_start(out=acat_sb[:], in_=acat_d[:, :])
            ident_sb = cpool.tile([P, P], F32)
            nc.sync.dma_start(out=ident_sb[:], in_=ident_d[:, :])

            # WA = W @ Acat  (so that a = x @ WA reuses the x^T tiles)
            wt_sb = []
            for j in range(2):
                wt_ps = ppool0.tile([P, P], F32, space="PSUM")
                nc.tensor.transpose(out=wt_ps[:], in_=w_sb[:, j * P:(j + 1) * P],
                                    identity=ident_sb[:])
                t = cpool.tile([P, P], F32, tag="wt")
                nc.vector.tensor_copy(out=t[:], in_=wt_ps[:])
                wt_sb.append(t)
            wa_ps = ppool0.tile([P, 2 * HEADS], F32, space="PSUM")
            nc.tensor.matmul(out=wa_ps[:], lhsT=wt_sb[0][:],
                             rhs=acat_sb[:, :2 * HEADS],
                             start=True, stop=False)
            nc.tensor.matmul(out=wa_ps[:], lhsT=wt_sb[1][:],
                             rhs=acat_sb[:, 2 * HEADS:],
                             start=False, stop=True)
            wa_sb = cpool.tile([P, 2 * HEADS], F32)
            nc.vector.tensor_copy(out=wa_sb[:], in_=wa_ps[:])

            for i in range(nx):
                r0 = i * P
                r1 = min(r0 + P, n_nodes)
                nr = r1 - r0
                xt = spool.tile([P, P], F32, tag="xt")
                if nr < P:
                    nc.vector.memset(xt[:], 0.0)
                nc.sync.dma_start(out=xt[:nr, :], in_=x_d[r0:r1, :])
                xT_ps = ppool.tile([P, P], F32, space="PSUM", tag="xT_ps")
                nc.tensor.transpose(out=xT_ps[:], in_=xt[:], identity=ident_sb[:])
                xT = spool.tile([P, P], F32, tag="xT")
                nc.vector.tensor_copy(out=xT[:], in_=xT_ps[:])
                h_ps = ppool.tile([P, ROW], F32, space="PSUM", tag="h_ps")
                nc.tensor.matmul(out=h_ps[:, 0:HC], lhsT=xT[:],
                                 rhs=w_sb[:], start=True, stop=True)
                nc.tensor.matmul(out=h_ps[:, HC:ROW], lhsT=xT[:],
                                 rhs=wa_sb[:], start=True, stop=True)
                stage = spool.tile([P, ROW], F32, tag="stage")
                nc.vector.tensor_copy(out=stage[:], in_=h_ps[:])
                nc.sync.dma_start(out=ha_d[r0:r0 + P, :], in_=stage[:])

    # ---------------- Phase B: per-group edge aggregation ----------------
    n_chunks = math.ceil(T / gather_chunk_tiles)
    with tile.TileContext(nc) as tc:
        with (
            tc.tile_pool(name="bconst", bufs=1) as cpool,
            tc.tile_pool(name="bsb", bufs=2) as gpool,
            tc.tile_pool(name="bsmall", bufs=3) as spool,
            tc.tile_pool(name="bs_s", bufs=2 * T) as s_pool,
            tc.tile_pool(name="bps", bufs=2, space="PSUM") as opool,
            tc.tile_pool(name="bps_st", bufs=3, space="PSUM") as stpool,
        ):
            iota_sb = cpool.tile([P, P], F32)
            nc.sync.dma_start(out=iota_sb[:], in_=iota_d[:, :])
            ident_sb = cpool.tile([P, P], F32)
            nc.sync.dma_start(out=ident_sb[:], in_=ident_d[:, :])
            bias_sb = cpool.tile([P, OUT_C], F32)
            nc.sync.dma_start(out=bias_sb[:], in_=bias_d[:, :])
            gidx_sb = cpool.tile([P, G * T], I32)
            nc.sync.dma_start(out=gidx_sb[:], in_=gidx_d[:, :])
            dstl_sb = cpool.tile([P, G * T], F32)
            nc.sync.dma_start(out=dstl_sb[:], in_=dstl_d[:, :])
            nidx_sb = cpool.tile([P, G], I32)
            nc.sync.dma_start(out=nidx_sb[:], in_=nidx_d[:, :])

            for g in range(G):
                gath = gpool.tile([P, T, ROW], F32, tag="gath")
                for t in range(T):
                    nc.gpsimd.indirect_dma_start(
                        out=gath[:, t, :],
                        out_offset=None,
                        in_=ha_d[:, :],
                        in_offset=bass.IndirectOffsetOnAxis(
                            ap=gidx_sb[:, g * T + t:g * T + t + 1], axis=0),
                    )
                adst_blk = spool.tile([P, 2 * HEADS], F32, tag="adst_blk")
                # gather a_dst rows of this group's nodes (cols 264:272 of ha)
                nc.gpsimd.indirect_dma_start(
                    out=adst_blk[:, 0:HEADS],
                    out_offset=None,
                    in_=ha_d[:, :],
                    in_offset=bass.IndirectOffsetOnAxis(
                        ap=nidx_sb[:, g:g + 1], axis=0),
                    element_offset=HC + HEADS,
                )

                out_ps = opool.tile([P, ROW - HEADS], F32, space="PSUM",
                                    tag="out_ps")       # [128, 264]
                adst_ps = opool.tile([P, T, HEADS], F32, space="PSUM",
                                     tag="adst_ps")
                s_tiles = []
                for t in range(T):
                    s_t = s_pool.tile([P, P], F32, tag="s")
                    nc.vector.tensor_tensor(
                        out=s_t[:],
                        in0=dstl_sb[:, g * T + t:g * T + t + 1].to_broadcast([P, P]),
                        in1=iota_sb[:],
                        op=mybir.AluOpType.is_equal,
                    )
                    st_ps = stpool.tile([P, P], F32, space="PSUM", tag="st_ps")
                    nc.tensor.transpose(out=st_ps[:], in_=s_t[:],
                                        identity=ident_sb[:])
                    st_sb = spool.tile([P, P], F32, tag="st_sb")
                    nc.vector.tensor_copy(out=st_sb[:], in_=st_ps[:])
                    nc.tensor.matmul(out=adst_ps[:, t, :],
                                     lhsT=st_sb[:], rhs=adst_blk[:, 0:HEADS],
                                     start=True, stop=True)
                    s_tiles.append(s_t)

                # ex = exp(leaky_relu(a_src[src] + a_dst[dst]))  for all T tiles
                s_att = spool.tile([P, T, HEADS], F32, tag="s_att")
                nc.vector.tensor_tensor(out=s_att[:],
                                        in0=gath[:, :, HC:HC + HEADS],
                                        in1=adst_ps[:],
                                        op=mybir.AluOpType.add)
                sa_s = spool.tile([P, T, HEADS], F32, tag="sa_s")
                nc.vector.tensor_scalar_mul(out=sa_s[:], in0=s_att[:],
                                            scalar1=NEG_SLOPE)
                u_att = spool.tile([P, T, HEADS], F32, tag="u_att")
                nc.vector.tensor_tensor(out=u_att[:], in0=s_att[:], in1=sa_s[:],
                                        op=mybir.AluOpType.max)
                ex = spool.tile([P, T, HEADS], F32, tag="ex")
                nc.scalar.activation(out=ex[:], in_=u_att[:],
                                     func=mybir.ActivationFunctionType.Exp)

                for t in range(T):
                    m2 = spool.tile([P, ROW - HEADS], F32, tag="m2")
                    nc.vector.tensor_tensor(
                        out=m2[:, 0:HC].rearrange("p (h c) -> p h c", h=HEADS),
                        in0=gath[:, t, 0:HC].rearrange("p (h c) -> p h c", h=HEADS),
                        in1=ex[:, t, :].unsqueeze(2).to_broadcast([P, HEADS, OUT_C]),
                        op=mybir.AluOpType.mult,
                    )
                    nc.vector.tensor_copy(out=m2[:, HC:HC + HEADS], in_=ex[:, t, :])
                    nc.tensor.matmul(out=out_ps[:], lhsT=s_tiles[t][:],
                                     rhs=m2[:], start=(t == 0),
                                     stop=(t == T - 1))

                # normalize + head mean + bias
                dr = spool.tile([P, HEADS], F32, tag="dr")
                nc.vector.tensor_scalar(out=dr[:], in0=out_ps[:, HC:HC + HEADS],
                                        scalar1=float(HEADS), scalar2=DENOM_EPS,
                                        op0=mybir.AluOpType.mult,
                                        op1=mybir.AluOpType.add)
                rcp = spool.tile([P, HEADS], F32, tag="rcp")
                nc.vector.reciprocal(out=rcp[:], in_=dr[:])
                o_n = spool.tile([P, HC], F32, tag="o_n")
                nc.vector.tensor_tensor(
                    out=o_n[:].rearrange("p (h c) -> p h c", h=HEADS),
                    in0=out_ps[:, 0:HC].rearrange("p (h c) -> p h c", h=HEADS),
                    in1=rcp[:].unsqueeze(2).to_broadcast([P, HEADS, OUT_C]),
                    op=mybir.AluOpType.mult,
                )
                t1_ = spool.tile([P, P], F32, tag="t1")
                nc.vector.tensor_tensor(out=t1_[:], in0=o_n[:, 0:P],
                                        in1=o_n[:, P:2 * P],
                                        op=mybir.AluOpType.add)
                t2_ = spool.tile([P, 2 * OUT_C], F32, tag="t2")
                nc.vector.tensor_tensor(out=t2_[:], in0=t1_[:, 0:2 * OUT_C],
                                        in1=t1_[:, 2 * OUT_C:P],
                                        op=mybir.AluOpType.add)
                o_f = spool.tile([P, OUT_C], F32, tag="o_f")
                nc.vector.tensor_tensor(out=o_f[:], in0=t2_[:, 0:OUT_C],
                                        in1=t2_[:, OUT_C:2 * OUT_C],
                                        op=mybir.AluOpType.add)
                o_b = spool.tile([P, OUT_C], F32, tag="o_b")
                nc.vector.tensor_tensor(out=o_b[:], in0=o_f[:], in1=bias_sb[:],
                                        op=mybir.AluOpType.add)
                nc.sync.dma_start(out=out_d[g * P:(g + 1) * P, :], in_=o_b[:])

    return nc


# ----------------------------------------------------------------------------
# Full kernel: host prep -> run on 8 cores -> unshard
# ----------------------------------------------------------------------------

def make_in_maps(x, W, att_src, att_dst, bias, plan, n_cores=8):
    consts = host_constants(W, att_src, att_dst, bias)
    x = np.ascontiguousarray(np.asarray(x, dtype=np.float32))
    in_maps = []
    for c in range(n_cores):
        m = dict(x=x, w=consts["w"], acat2=consts["acat2"],
                 ident=consts["ident"], iota_f=consts["iota_f"],
                 bias_rep=consts["bias_rep"],
                 gidx=plan["gidx"][c], dstl=plan["dstl"][c],
                 nidx=plan["nidx"][c])
        in_maps.append(m)
    return in_maps


def unshard(results, plan, n_nodes):
    out = np.zeros((n_nodes, OUT_C), dtype=np.float32)
    node_of = plan["node_of"]
    n_cores, G, _ = node_of.shape
    for c in range(n_cores):
        o = results[c]["out"].reshape(G, P, OUT_C)
        for g in range(G):
            mask = node_of[c, g] >= 0
            if mask.any():
                out[node_of[c, g, mask]] = o[g, mask]
    return out




# ----------------------------------------------------------------------------
# Walrus in this container accepts at most ONE semaphore wait per engine
# instruction.  Rebuild blocks, hoisting extra waits onto NOP carriers
# placed immediately before the instruction (same engine) — semantically
# identical (the engine just stalls one instruction earlier).
# ----------------------------------------------------------------------------

def _engine_obj(nc, engine):
    return {
        mb.EngineType.PE: nc.tensor,
        mb.EngineType.DVE: nc.vector,
        mb.EngineType.Activation: nc.scalar,
        mb.EngineType.SP: nc.sync,
        mb.EngineType.Pool: nc.gpsimd,
    }[engine]


def legalize_waits(nc, max_waits=1):
    Op = nc.isa.Opcode
    for f in nc.m.functions:
        new_blocks = []
        for blk in f.blocks:
            out = []
            for inst in blk.instructions:
                si = inst.sync_info
                waits = list(si.on_wait) if si is not None else []
                if len(waits) > max_waits:
                    eng = _engine_obj(nc, inst.engine)
                    extra, keep = waits[:-max_waits], waits[-max_waits:]
                    opc = (Op.NEURON_ISA_TPB_OPCODE_ENGINE_NOP
                           if inst.engine == mb.EngineType.Pool
                           else Op.NEURON_ISA_TPB_OPCODE_NOP)
                    for w in extra:
                        nop = eng._isa(opc, {})
                        nop.sync_info = mb.SyncInfo(on_wait=[w], on_update=[])
                        out.append(nop)
                    inst.sync_info = mb.SyncInfo(on_wait=keep,
                                                 on_update=list(si.on_update))
                out.append(inst)
            new_blocks.append(mb.BasicBlock(
                name=blk.name, instructions=out,
                IsPredicated=blk.IsPredicated, IsExit=blk.IsExit,
                IsLoopEntry=blk.IsLoopEntry))
        f.blocks = new_blocks
    return nc


_CACHE = {}


def kernel(x, edge_index, batch, W, att_src, att_dst, bias):
    x = np.ascontiguousarray(np.asarray(x, dtype=np.float32))
    n_nodes = x.shape[0]
    plan = plan_groups(np.asarray(edge_index), n_nodes, 17)
    key = (n_nodes, plan["G"], plan["T"])
    if key not in _CACHE:
        nc = build_bass(n_nodes, plan["G"], plan["T"])
        legalize_waits(nc)
        _CACHE[key] = nc
    nc = _CACHE[key]
    in_maps = make_in_maps(x, W, att_src, att_dst, bias, plan)
    from concourse.bass_utils import run_bass_kernel_spmd
    res = run_bass_kernel_spmd(nc, in_maps, list(range(8)), trace=False)
    return unshard(res.results, plan, n_nodes)



# revision 4
# speedup vs baseline: 2.2431x; 2.2431x over previous
"""Self-contained GAT (PyG GATConv, concat=False) Bass/Tile kernel for 8
Trainium2 NeuronCores.  kernel(**inputs) takes the full-graph inputs and
returns the full [N, 32] output.

Strategy (dst-partition): nodes are sorted by in-degree (self-loops included)
and packed into groups of 128; inside a group the PARTITION index is the
destination node, and the free dim holds that node's incoming edges
(T = group max degree slots per node).  This removes the one-hot scatter
matrices entirely: segment-sum over a group's edges is T identity-matmuls
accumulated in PSUM.  Groups are snake-dealt to the 8 cores; all cores run
one shared program (per-group T schedule identical), per-core differences
live in the data (a per-core node permutation makes each core's own groups
occupy rows [g*128,(g+1)*128) of its private `ha` scratch).

Per core:
  Phase A: ha[r] = [h_cmajor(256) | a_src(8) | a_dst(8)] fp16 for ALL rows
           (x pre-transposed+fp16 on host, one 272-col fp16 matmul per
           128-row tile; PSUM->SBUF cast copies split ACT/DVE).
  Phase B per group: one multi-row indirect DMA gathers all T*128 edge
           source rows (264 cols); a_dst of the group's own nodes comes from
           a strided slice load; attention = exp(leaky(a_src+a_dst) - 8)
           computed in-place (DVE leaky, ACT exp, shift avoids fp16
           overflow; softmax is shift-invariant so no max pass is needed);
           messages multiply in fp16 2x mode (head-minor layout keeps the
           broadcast packed); T identity-matmuls accumulate [128,264] in
           PSUM (256 msg cols + 8 denominator cols); normalize + head-mean
           + bias on DVE; store [128,32].
Empty edge slots gather a pad row with a_src=-60000 so their exp underflows
to exactly 0 in fp16.
"""

import math

import numpy as np

import concourse.tile as tile
import concourse.mybir as mb
from concourse import bass, mybir

P = 128
F32 = mybir.dt.float32
F16 = mybir.dt.float16
I32 = mybir.dt.int32

HEADS = 8
OUT_C = 32
HC = HEADS * OUT_C          # 256
ROW = HC + 2 * HEADS        # 272 per ha row: h(c-major) | a_src | a_dst
GROW = HC + HEADS           # 264 gathered cols per edge row
NEG_SLOPE = 0.2
EXP_SHIFT = -8.0            # exp(s - 8): keeps fp16 in range; shift-invariant
DENOM_EPS = 1e-30
PAD_ASRC = -60000.0         # pad-slot a_src: exp underflows to exact 0
N_CORES = 8
CHUNK = 4                   # phase-A node tiles per DMA chunk


# ----------------------------------------------------------------------------
# Host-side planning
# ----------------------------------------------------------------------------

def plan_dst(edge_index: np.ndarray, n_nodes: int, n_cores: int = N_CORES):
    src = np.asarray(edge_index[0], dtype=np.int64)
    dst = np.asarray(edge_index[1], dtype=np.int64)
    loops = np.arange(n_nodes, dtype=np.int64)
    src = np.concatenate([src, loops])
    dst = np.concatenate([dst, loops])

    deg = np.bincount(dst, minlength=n_nodes).astype(np.int64)   # >= 1
    node_order = np.argsort(-deg, kind="stable")                 # desc degree
    node_rank = np.empty(n_nodes, dtype=np.int64)
    node_rank[node_order] = np.arange(n_nodes)

    n_groups = math.ceil(n_nodes / P)
    n_groups = math.ceil(n_groups / n_cores) * n_cores           # e.g. 392
    G = n_groups // n_cores
    NROWS = n_groups * P
    PADROW = NROWS

    # edges sorted by dst; per-dst CSR and within-dst counter
    order_e = np.argsort(dst, kind="stable")
    src_s = src[order_e]
    dst_s = dst[order_e]
    csr = np.zeros(n_nodes + 1, dtype=np.int64)
    np.cumsum(deg, out=csr[1:])
    t_of_edge = np.arange(len(dst_s)) - csr[dst_s]

    # per-rank-group T; slot j serves ranks [8j, 8j+8) => T_slot = Tg[8j]
    Tg = np.ones(n_groups, dtype=np.int64)
    for k in range(n_groups):
        if k * P < n_nodes:
            Tg[k] = deg[node_order[k * P]]
    T_slot = Tg[n_cores * np.arange(G)].astype(np.int64)
    offs = np.zeros(G + 1, dtype=np.int64)
    np.cumsum(T_slot, out=offs[1:])
    sumT = int(offs[-1])

    # snake deal: slot j, core c -> rank 8j + (c if j even else 7-c)
    rank_core = np.empty(n_groups, dtype=np.int64)
    rank_slot = np.empty(n_groups, dtype=np.int64)
    for j in range(G):
        for c in range(n_cores):
            r = n_cores * j + (c if j % 2 == 0 else n_cores - 1 - c)
            rank_core[r] = c
            rank_slot[r] = j

    # per-node placement
    n_core = rank_core[node_rank // P]          # owning core of each node
    n_slot = rank_slot[node_rank // P]          # group slot on that core
    n_part = node_rank % P                      # partition within group

    # per-core node permutation: own groups first (slot-major), then the rest
    perms = []       # perms[c][row] = node id or -1
    pos = []         # pos[c][node] = row of node in core c's ha
    node_of = []     # node_of[c][G*P] = node id or -1 (for unshard)
    for c in range(n_cores):
        perm = np.full(NROWS, -1, dtype=np.int64)
        own = n_core == c
        own_rows = n_slot[own] * P + n_part[own]
        own_nodes = np.where(own)[0]
        perm[own_rows] = own_nodes
        rest = np.where(~own)[0]
        perm[G * P:G * P + len(rest)] = rest
        p = np.empty(n_nodes, dtype=np.int64)
        rows = np.where(perm >= 0)[0]
        p[perm[rows]] = rows
        perms.append(perm)
        pos.append(p)
        node_of.append(perm[:G * P].copy())

    # per-core gather index maps
    gidx = np.full((n_cores, P, sumT), PADROW, dtype=np.int32)
    e_core = n_core[dst_s]
    e_col = offs[n_slot[dst_s]] + t_of_edge
    e_part = n_part[dst_s]
    for c in range(n_cores):
        m = e_core == c
        gidx[c, e_part[m], e_col[m]] = pos[c][src_s[m]].astype(np.int32)

    return dict(G=G, T_slot=T_slot.tolist(), offs=offs, sumT=sumT,
                NROWS=NROWS, PADROW=PADROW, perms=perms, node_of=node_of,
                gidx=gidx)


def host_constants(W, att_src, att_dst, bias):
    W = np.asarray(W, dtype=np.float32)          # [128, 256]
    att_src = np.asarray(att_src, dtype=np.float32)
    att_dst = np.asarray(att_dst, dtype=np.float32)
    Wr = W.reshape(P, HEADS, OUT_C)
    wcat = np.zeros((P, ROW), dtype=np.float32)
    wcat[:, :HC] = Wr.transpose(0, 2, 1).reshape(P, HC)       # col = c*8+h
    wcat[:, HC:HC + HEADS] = np.einsum("khc,hc->kh", Wr, att_src)
    wcat[:, HC + HEADS:] = np.einsum("khc,hc->kh", Wr, att_dst)
    padrow = np.zeros((1, ROW), dtype=np.float16)
    padrow[0, HC:HC + HEADS] = PAD_ASRC
    ident = np.eye(P, dtype=np.float16)
    bias_rep = np.tile(np.asarray(bias, dtype=np.float32)[None, :], (P, 1))
    return dict(wcat=wcat.astype(np.float16), padrow=padrow, ident=ident,
                bias_rep=bias_rep)


# ----------------------------------------------------------------------------
# Bass program (shared by all cores)
# ----------------------------------------------------------------------------

def build_bass(G: int, T_slot, sumT: int, NROWS: int):
    nc = bass.Bass(trn_type="TRN2", dynamic_dma_scratch_size=65536)

    xt_d = nc.dram_tensor("xt", [P, NROWS], F16, kind="ExternalInput")
    wcat_d = nc.dram_tensor("wcat", [P, ROW], F16, kind="ExternalInput")
    pad_d = nc.dram_tensor("padrow", [1, ROW], F16, kind="ExternalInput")
    ident_d = nc.dram_tensor("ident", [P, P], F16, kind="ExternalInput")
    bias_d = nc.dram_tensor("bias_rep", [P, OUT_C], F32, kind="ExternalInput")
    gidx_d = nc.dram_tensor("gidx", [P, sumT], I32, kind="ExternalInput")
    out_d = nc.dram_tensor("out", [G * P, OUT_C], F32, kind="ExternalOutput")

    ha_d = nc.dram_tensor("ha", [NROWS + 1, ROW], F16)   # +1 pad row

    n_tiles = NROWS // P
    n_chunks = n_tiles // CHUNK

    # ---------------- Phase A: ha = [x@W | x@WAs | x@WAd] (fp16) ------------
    with tile.TileContext(nc) as tc:
        with (
            tc.tile_pool(name="aconst", bufs=1) as cpool,
            tc.tile_pool(name="asb", bufs=3) as spool,
            tc.tile_pool(name="aps", bufs=4, space="PSUM") as ppool,
        ):
            wcat_sb = cpool.tile([P, ROW], F16, name="wcat_sb")
            nc.sync.dma_start(out=wcat_sb[:], in_=wcat_d[:, :])
            pad_sb = cpool.tile([1, ROW], F16, name="pad_sb")
            nc.sync.dma_start(out=pad_sb[:], in_=pad_d[:, :])
            nc.sync.dma_start(out=ha_d[NROWS:NROWS + 1, :], in_=pad_sb[:])

            for i in range(n_chunks):
                c0 = i * CHUNK * P
                xt = spool.tile([P, CHUNK * P], F16, tag="xt")
                nc.sync.dma_start(out=xt[:], in_=xt_d[:, c0:c0 + CHUNK * P])
                stage = spool.tile([P, CHUNK, ROW], F16, tag="stage")
                for j in range(CHUNK):
                    hps = ppool.tile([P, ROW], F32, space="PSUM", tag="hps")
                    nc.tensor.matmul(out=hps[:], lhsT=xt[:, j * P:(j + 1) * P],
                                     rhs=wcat_sb[:], start=True, stop=True)
                    ti = i * CHUNK + j
                    eng = nc.vector if ti % 6 == 0 else nc.scalar
                    if eng is nc.vector:
                        eng.tensor_copy(out=stage[:, j, :], in_=hps[:])
                    else:
                        eng.activation(out=stage[:, j, :], in_=hps[:],
                                       func=mybir.ActivationFunctionType.Copy)
                nc.scalar.dma_start(
                    out=ha_d[c0:c0 + CHUNK * P, :].rearrange(
                        "(j p) c -> p j c", p=P),
                    in_=stage[:],
                )

    # ---------------- Phase B: per-group edge aggregation -------------------
    with tile.TileContext(nc) as tc:
        with (
            tc.tile_pool(name="bconst", bufs=1) as cpool,
            tc.tile_pool(name="bgath", bufs=2) as gpool,
            tc.tile_pool(name="bsmall", bufs=3) as spool,
            tc.tile_pool(name="bout", bufs=2) as opool,
            tc.tile_pool(name="bps", bufs=2, space="PSUM") as ppool,
        ):
            ident_sb = cpool.tile([P, P], F16, name="ident_sb")
            nc.sync.dma_start(out=ident_sb[:], in_=ident_d[:, :])
            bias_sb = cpool.tile([P, OUT_C], F32, name="bias_sb")
            nc.sync.dma_start(out=bias_sb[:], in_=bias_d[:, :])
            gidx_sb = cpool.tile([P, sumT], I32, name="gidx_sb")
            nc.sync.dma_start(out=gidx_sb[:], in_=gidx_d[:, :])
            shift_sb = cpool.tile([P, 1], F32, name="shift_sb")
            nc.vector.memset(shift_sb[:], EXP_SHIFT)

            off = 0
            for g in range(G):
                T = int(T_slot[g])
                gath = gpool.tile([P, T, GROW], F16, tag="gath")
                nc.gpsimd.indirect_dma_start(
                    out=gath[:, :, :],
                    out_offset=None,
                    in_=ha_d[:, :],
                    in_offset=bass.IndirectOffsetOnAxis(
                        ap=gidx_sb[:, off:off + T], axis=0),
                )
                adst = spool.tile([P, HEADS], F16, tag="adst")
                nc.sync.dma_start(out=adst[:],
                                  in_=ha_d[g * P:(g + 1) * P, HC + HEADS:ROW])

                att = gath[:, :, HC:GROW]
                # s = a_src + a_dst ; leaky = max(s, 0.2*s) ; ex = exp(leaky-8)
                nc.vector.tensor_tensor(
                    out=att, in0=att,
                    in1=adst[:].unsqueeze(1).to_broadcast([P, T, HEADS]),
                    op=mybir.AluOpType.add)
                sl = spool.tile([P, T, HEADS], F16, tag="sl")
                nc.vector.tensor_scalar_mul(out=sl[:], in0=att,
                                            scalar1=NEG_SLOPE)
                nc.vector.tensor_tensor(out=att, in0=att, in1=sl[:],
                                        op=mybir.AluOpType.max)
                nc.scalar.activation(out=att, in_=att,
                                     func=mybir.ActivationFunctionType.Exp,
                                     bias=shift_sb[:])
                # msg = h * alpha_unnorm (head-minor keeps broadcast packed)
                nc.vector.tensor_tensor(
                    out=gath[:, :, 0:HC].rearrange("p t (c h) -> p t c h",
                                                   h=HEADS),
                    in0=gath[:, :, 0:HC].rearrange("p t (c h) -> p t c h",
                                                   h=HEADS),
                    in1=att.unsqueeze(2).to_broadcast([P, T, OUT_C, HEADS]),
                    op=mybir.AluOpType.mult)

                ops = ppool.tile([P, GROW], F32, space="PSUM", tag="ops")
                for t in range(T):
                    nc.tensor.matmul(out=ops[:], lhsT=ident_sb[:],
                                     rhs=gath[:, t, :],
                                     start=(t == 0), stop=(t == T - 1))

                # normalize + head mean + bias
                dr = spool.tile([P, HEADS], F32, tag="dr")
                nc.vector.tensor_scalar(out=dr[:], in0=ops[:, HC:GROW],
                                        scalar1=float(HEADS),
                                        scalar2=DENOM_EPS,
                                        op0=mybir.AluOpType.mult,
                                        op1=mybir.AluOpType.add)
                rcp = spool.tile([P, HEADS], F32, tag="rcp")
                nc.vector.reciprocal(out=rcp[:], in_=dr[:])
                on = opool.tile([P, OUT_C, HEADS], F16, tag="on")
                nc.vector.tensor_tensor(
                    out=on[:],
                    in0=ops[:, 0:HC].rearrange("p (c h) -> p c h", h=HEADS),
                    in1=rcp[:].unsqueeze(1).to_broadcast([P, OUT_C, HEADS]),
                    op=mybir.AluOpType.mult)
                t1 = spool.tile([P, OUT_C, 4], F16, tag="t1")
                nc.vector.tensor_tensor(out=t1[:], in0=on[:, :, 0:4],
                                        in1=on[:, :, 4:8],
                                        op=mybir.AluOpType.add)
                t2 = spool.tile([P, OUT_C, 2], F16, tag="t2")
                nc.vector.tensor_tensor(out=t2[:], in0=t1[:, :, 0:2],
                                        in1=t1[:, :, 2:4],
                                        op=mybir.AluOpType.add)
                t3 = spool.tile([P, OUT_C], F16, tag="t3")
                nc.vector.tensor_tensor(out=t3[:],
                                        in0=t2[:, :, 0:1].rearrange(
                                            "p c one -> p (c one)"),
                                        in1=t2[:, :, 1:2].rearrange(
                                            "p c one -> p (c one)"),
                                        op=mybir.AluOpType.add)
                ob = opool.tile([P, OUT_C], F32, tag="ob")
                nc.vector.tensor_tensor(out=ob[:], in0=t3[:], in1=bias_sb[:],
                                        op=mybir.AluOpType.add)
                nc.sync.dma_start(out=out_d[g * P:(g + 1) * P, :], in_=ob[:])
                off += T

    return nc


# ----------------------------------------------------------------------------
# Walrus in this container accepts at most ONE semaphore wait per engine
# instruction.  Rebuild blocks, hoisting extra waits onto NOP carriers
# placed immediately before the instruction (same engine) — semantically
# identical (the engine just stalls one instruction earlier).
# ----------------------------------------------------------------------------

def _engine_obj(nc, engine):
    return {
        mb.EngineType.PE: nc.tensor,
        mb.EngineType.DVE: nc.vector,
        mb.EngineType.Activation: nc.scalar,
        mb.EngineType.SP: nc.sync,
        mb.EngineType.Pool: nc.gpsimd,
    }[engine]


def legalize_waits(nc, max_waits=1):
    Op = nc.isa.Opcode
    for f in nc.m.functions:
        new_blocks = []
        for blk in f.blocks:
            out = []
            for inst in blk.instructions:
                si = inst.sync_info
                waits = list(si.on_wait) if si is not None else []
                if len(waits) > max_waits:
                    eng = _engine_obj(nc, inst.engine)
                    extra, keep = waits[:-max_waits], waits[-max_waits:]
                    opc = (Op.NEURON_ISA_TPB_OPCODE_ENGINE_NOP
                           if inst.engine == mb.EngineType.Pool
                           else Op.NEURON_ISA_TPB_OPCODE_NOP)
                    for w in extra:
                        nop = eng._isa(opc, {})
                        nop.sync_info = mb.SyncInfo(on_wait=[w], on_update=[])
                        out.append(nop)
                    inst.sync_info = mb.SyncInfo(on_wait=keep,
                                                 on_update=list(si.on_update))
                out.append(inst)
            new_blocks.append(mb.BasicBlock(
                name=blk.name, instructions=out,
                IsPredicated=blk.IsPredicated, IsExit=blk.IsExit,
                IsLoopEntry=blk.IsLoopEntry))
        f.blocks = new_blocks
    return nc


# ----------------------------------------------------------------------------
# Full kernel: host prep -> run on 8 cores -> unshard
# ----------------------------------------------------------------------------

def make_in_maps(x, plan, consts, n_cores=N_CORES):
    x16 = np.asarray(x, dtype=np.float16)
    n = x16.shape[0]
    NROWS = plan["NROWS"]
    in_maps = []
    for c in range(n_cores):
        perm = plan["perms"][c]
        xp = np.zeros((NROWS, P), dtype=np.float16)
        rows = np.where(perm >= 0)[0]
        xp[rows] = x16[perm[rows]]
        m = dict(xt=np.ascontiguousarray(xp.T),
                 wcat=consts["wcat"], padrow=consts["padrow"],
                 ident=consts["ident"], bias_rep=consts["bias_rep"],
                 gidx=plan["gidx"][c])
        in_maps.append(m)
    return in_maps


def unshard(results, plan, n_nodes):
    out = np.zeros((n_nodes, OUT_C), dtype=np.float32)
    for c in range(N_CORES):
        no = plan["node_of"][c]
        res = results[c]["out"]
        mask = no >= 0
        out[no[mask]] = res[mask]
    return out


_CACHE = {}


def kernel(x, edge_index, batch, W, att_src, att_dst, bias):
    x = np.ascontiguousarray(np.asarray(x, dtype=np.float32))
    n_nodes = x.shape[0]
    plan = plan_dst(np.asarray(edge_index), n_nodes)
    key = (n_nodes, plan["G"], tuple(plan["T_slot"]), plan["NROWS"])
    if key not in _CACHE:
        nc = build_bass(plan["G"], plan["T_slot"], plan["sumT"],
                        plan["NROWS"])
        legalize_waits(nc)
        _CACHE[key] = nc
    nc = _CACHE[key]
    consts = host_constants(W, att_src, att_dst, bias)
    in_maps = make_in_maps(x, plan, consts)
    from concourse.bass_utils import run_bass_kernel_spmd
    res = run_bass_kernel_spmd(nc, in_maps, list(range(N_CORES)), trace=False)
    return unshard(res.results, plan, n_nodes)


# revision 7
# speedup vs baseline: 3.1749x; 1.4154x over previous
"""Self-contained GAT (PyG GATConv, concat=False) Bass/Tile kernel for 8
Trainium2 NeuronCores.  kernel(**inputs) takes the full-graph inputs and
returns the full [N, 32] output.

Strategy (dst-partition): nodes are sorted by in-degree (self-loops included)
and packed into groups of 128; inside a group the PARTITION index is the
destination node, and the free dim holds that node's incoming edges
(T = group max degree slots per node).  This removes the one-hot scatter
matrices entirely: segment-sum over a group's edges is T identity-matmuls
accumulated in PSUM.  Groups are snake-dealt to the 8 cores; all cores run
one shared program (per-group T schedule identical), per-core differences
live in the data (a per-core node permutation makes each core's own groups
occupy rows [g*128,(g+1)*128) of its private `ha` scratch).

Per core:
  Phase A: ha[r] = [h_cmajor(256) | a_src(8) | a_dst(8)] fp16 for ALL rows
           (x pre-transposed+fp16 on host, one 272-col fp16 matmul per
           128-row tile; PSUM->SBUF cast copies split ACT/DVE).
  Phase B per group: one multi-row indirect DMA gathers all T*128 edge
           source rows (264 cols); a_dst of the group's own nodes comes from
           a strided slice load; attention = exp(leaky(a_src+a_dst) - 8)
           computed in-place (DVE leaky, ACT exp, shift avoids fp16
           overflow; softmax is shift-invariant so no max pass is needed);
           messages multiply in fp16 2x mode (head-minor layout keeps the
           broadcast packed); T identity-matmuls accumulate [128,264] in
           PSUM (256 msg cols + 8 denominator cols); normalize + head-mean
           + bias on DVE; store [128,32].
Empty edge slots gather a pad row with a_src=-60000 so their exp underflows
to exactly 0 in fp16.
"""

import math

import numpy as np

import concourse.tile as tile
import concourse.mybir as mb
from concourse import bass, mybir

P = 128
F32 = mybir.dt.float32
F16 = mybir.dt.float16
I32 = mybir.dt.int32

HEADS = 8
OUT_C = 32
HC = HEADS * OUT_C          # 256
ROW = HC + 2 * HEADS        # 272 per ha row: h(c-major) | a_src | a_dst
GROW = HC + HEADS           # 264 gathered cols per edge row
NEG_SLOPE = 0.2
EXP_SHIFT = -8.0            # exp(s - 8): keeps fp16 in range; shift-invariant
DENOM_EPS = 1e-30
PAD_ASRC = -60000.0         # pad-slot a_src: exp underflows to exact 0
N_CORES = 8
CHUNK = 4                   # phase-A node tiles per DMA chunk


# ----------------------------------------------------------------------------
# Host-side planning
# ----------------------------------------------------------------------------

def plan_dst(edge_index: np.ndarray, n_nodes: int, n_cores: int = N_CORES):
    src = np.asarray(edge_index[0], dtype=np.int64)
    dst = np.asarray(edge_index[1], dtype=np.int64)
    loops = np.arange(n_nodes, dtype=np.int64)
    src = np.concatenate([src, loops])
    dst = np.concatenate([dst, loops])

    deg = np.bincount(dst, minlength=n_nodes).astype(np.int64)   # >= 1
    node_order = np.argsort(-deg, kind="stable")                 # desc degree
    node_rank = np.empty(n_nodes, dtype=np.int64)
    node_rank[node_order] = np.arange(n_nodes)

    n_groups = math.ceil(n_nodes / P)
    n_groups = math.ceil(n_groups / n_cores) * n_cores           # e.g. 392
    G = n_groups // n_cores
    NROWS = n_groups * P
    PADROW = NROWS

    # edges sorted by dst; per-dst CSR and within-dst counter
    order_e = np.argsort(dst, kind="stable")
    src_s = src[order_e]
    dst_s = dst[order_e]
    csr = np.zeros(n_nodes + 1, dtype=np.int64)
    np.cumsum(deg, out=csr[1:])
    t_of_edge = np.arange(len(dst_s)) - csr[dst_s]

    # per-rank-group T; slot j serves ranks [8j, 8j+8) => T_slot = Tg[8j]
    Tg = np.ones(n_groups, dtype=np.int64)
    for k in range(n_groups):
        if k * P < n_nodes:
            Tg[k] = deg[node_order[k * P]]
    T_slot = Tg[n_cores * np.arange(G)].astype(np.int64)
    offs = np.zeros(G + 1, dtype=np.int64)
    np.cumsum(T_slot, out=offs[1:])
    sumT = int(offs[-1])

    # snake deal: slot j, core c -> rank 8j + (c if j even else 7-c)
    rank_core = np.empty(n_groups, dtype=np.int64)
    rank_slot = np.empty(n_groups, dtype=np.int64)
    for j in range(G):
        for c in range(n_cores):
            r = n_cores * j + (c if j % 2 == 0 else n_cores - 1 - c)
            rank_core[r] = c
            rank_slot[r] = j

    # per-node placement
    n_core = rank_core[node_rank // P]          # owning core of each node
    n_slot = rank_slot[node_rank // P]          # group slot on that core
    n_part = node_rank % P                      # partition within group

    # per-core node permutation: own groups first (slot-major), then the rest
    perms = []       # perms[c][row] = node id or -1
    pos = []         # pos[c][node] = row of node in core c's ha
    node_of = []     # node_of[c][G*P] = node id or -1 (for unshard)
    for c in range(n_cores):
        perm = np.full(NROWS, -1, dtype=np.int64)
        own = n_core == c
        own_rows = n_slot[own] * P + n_part[own]
        own_nodes = np.where(own)[0]
        perm[own_rows] = own_nodes
        rest = np.where(~own)[0]
        perm[G * P:G * P + len(rest)] = rest
        p = np.empty(n_nodes, dtype=np.int64)
        rows = np.where(perm >= 0)[0]
        p[perm[rows]] = rows
        perms.append(perm)
        pos.append(p)
        node_of.append(perm[:G * P].copy())

    # per-core gather index maps
    gidx = np.full((n_cores, P, sumT), PADROW, dtype=np.int32)
    e_core = n_core[dst_s]
    e_col = offs[n_slot[dst_s]] + t_of_edge
    e_part = n_part[dst_s]
    for c in range(n_cores):
        m = e_core == c
        gidx[c, e_part[m], e_col[m]] = pos[c][src_s[m]].astype(np.int32)

    return dict(G=G, T_slot=T_slot.tolist(), offs=offs, sumT=sumT,
                NROWS=NROWS, PADROW=PADROW, perms=perms, node_of=node_of,
                gidx=gidx)


def host_constants(W, att_src, att_dst, bias):
    W = np.asarray(W, dtype=np.float32)          # [128, 256]
    att_src = np.asarray(att_src, dtype=np.float32)
    att_dst = np.asarray(att_dst, dtype=np.float32)
    Wr = W.reshape(P, HEADS, OUT_C)
    wcat = np.zeros((P, ROW), dtype=np.float32)
    wcat[:, :HC] = Wr.transpose(0, 2, 1).reshape(P, HC)       # col = c*8+h
    wcat[:, HC:HC + HEADS] = np.einsum("khc,hc->kh", Wr, att_src)
    wcat[:, HC + HEADS:] = np.einsum("khc,hc->kh", Wr, att_dst)
    padrow = np.zeros((1, ROW), dtype=np.float16)
    padrow[0, HC:HC + HEADS] = PAD_ASRC
    ident = np.eye(P, dtype=np.float16)
    bias_rep = np.tile(np.asarray(bias, dtype=np.float32)[None, :], (P, 1))
    return dict(wcat=wcat.astype(np.float16), padrow=padrow, ident=ident,
                bias_rep=bias_rep)


# ----------------------------------------------------------------------------
# Bass program (shared by all cores)
# ----------------------------------------------------------------------------

def build_bass(G: int, T_slot, sumT: int, NROWS: int):
    nc = bass.Bass(trn_type="TRN2", dynamic_dma_scratch_size=65536)

    xt_d = nc.dram_tensor("xt", [P, NROWS], F16, kind="ExternalInput")
    wcat_d = nc.dram_tensor("wcat", [P, ROW], F16, kind="ExternalInput")
    pad_d = nc.dram_tensor("padrow", [1, ROW], F16, kind="ExternalInput")
    ident_d = nc.dram_tensor("ident", [P, P], F16, kind="ExternalInput")
    bias_d = nc.dram_tensor("bias_rep", [P, OUT_C], F32, kind="ExternalInput")
    gidx_d = nc.dram_tensor("gidx", [P, sumT], I32, kind="ExternalInput")
    out_d = nc.dram_tensor("out", [G * P, OUT_C], F32, kind="ExternalOutput")

    ha_d = nc.dram_tensor("ha", [NROWS + 1, ROW], F16)   # +1 pad row

    n_tiles = NROWS // P
    n_chunks = n_tiles // CHUNK

    # ---------------- Phase A: ha = [x@W | x@WAs | x@WAd] (fp16) ------------
    with tile.TileContext(nc) as tc:
        with (
            tc.tile_pool(name="aconst", bufs=1) as cpool,
            tc.tile_pool(name="asb", bufs=3) as spool,
            tc.tile_pool(name="aps", bufs=4, space="PSUM") as ppool,
        ):
            wcat_sb = cpool.tile([P, ROW], F16, name="wcat_sb")
            nc.sync.dma_start(out=wcat_sb[:], in_=wcat_d[:, :])
            pad_sb = cpool.tile([1, ROW], F16, name="pad_sb")
            nc.sync.dma_start(out=pad_sb[:], in_=pad_d[:, :])
            nc.sync.dma_start(out=ha_d[NROWS:NROWS + 1, :], in_=pad_sb[:])

            for i in range(n_chunks):
                c0 = i * CHUNK * P
                xt = spool.tile([P, CHUNK * P], F16, tag="xt")
                nc.sync.dma_start(out=xt[:], in_=xt_d[:, c0:c0 + CHUNK * P])
                stage = spool.tile([P, CHUNK, ROW], F16, tag="stage")
                for j in range(CHUNK):
                    hps = ppool.tile([P, ROW], F32, space="PSUM", tag="hps")
                    nc.tensor.matmul(out=hps[:], lhsT=xt[:, j * P:(j + 1) * P],
                                     rhs=wcat_sb[:], start=True, stop=True)
                    ti = i * CHUNK + j
                    if ti % 2 == 0:
                        nc.vector.tensor_copy(out=stage[:, j, :], in_=hps[:])
                    else:
                        nc.scalar.activation(
                            out=stage[:, j, :], in_=hps[:],
                            func=mybir.ActivationFunctionType.Copy)
                nc.scalar.dma_start(
                    out=ha_d[c0:c0 + CHUNK * P, :].rearrange(
                        "(j p) c -> p j c", p=P),
                    in_=stage[:],
                )

    # ---------------- Phase B: per-group edge aggregation -------------------
    with tile.TileContext(nc) as tc:
        with (
            tc.tile_pool(name="bconst", bufs=1) as cpool,
            tc.tile_pool(name="bgath", bufs=3) as gpool,
            tc.tile_pool(name="bsmall", bufs=4) as spool,
            tc.tile_pool(name="bout", bufs=3) as opool,
            tc.tile_pool(name="bps", bufs=4, space="PSUM") as ppool,
        ):
            ident_sb = cpool.tile([P, P], F16, name="ident_sb")
            nc.sync.dma_start(out=ident_sb[:], in_=ident_d[:, :])
            bias_sb = cpool.tile([P, OUT_C], F32, name="bias_sb")
            nc.sync.dma_start(out=bias_sb[:], in_=bias_d[:, :])
            gidx_sb = cpool.tile([P, sumT], I32, name="gidx_sb")
            nc.sync.dma_start(out=gidx_sb[:], in_=gidx_d[:, :])
            shift_sb = cpool.tile([P, 1], F32, name="shift_sb")
            nc.vector.memset(shift_sb[:], EXP_SHIFT)

            off = 0
            for g in range(G):
                T = int(T_slot[g])
                gath = gpool.tile([P, T, GROW], F16, tag="gath")
                nc.gpsimd.indirect_dma_start(
                    out=gath[:, :, :],
                    out_offset=None,
                    in_=ha_d[:, :],
                    in_offset=bass.IndirectOffsetOnAxis(
                        ap=gidx_sb[:, off:off + T], axis=0),
                )
                adst = spool.tile([P, HEADS], F16, tag="adst")
                nc.sync.dma_start(out=adst[:],
                                  in_=ha_d[g * P:(g + 1) * P, HC + HEADS:ROW])

                att = gath[:, :, HC:GROW]
                # s = a_src + a_dst ; leaky = max(s, 0.2*s) ; ex = exp(leaky-8)
                nc.vector.tensor_tensor(
                    out=att, in0=att,
                    in1=adst[:].unsqueeze(1).to_broadcast([P, T, HEADS]),
                    op=mybir.AluOpType.add)
                sl = spool.tile([P, T, HEADS], F16, tag="sl")
                nc.vector.tensor_scalar_mul(out=sl[:], in0=att,
                                            scalar1=NEG_SLOPE)
                nc.vector.tensor_tensor(out=att, in0=att, in1=sl[:],
                                        op=mybir.AluOpType.max)
                nc.scalar.activation(out=att, in_=att,
                                     func=mybir.ActivationFunctionType.Exp,
                                     bias=shift_sb[:])
                # msg = h * alpha_unnorm (head-minor keeps broadcast packed)
                nc.vector.tensor_tensor(
                    out=gath[:, :, 0:HC].rearrange("p t (c h) -> p t c h",
                                                   h=HEADS),
                    in0=gath[:, :, 0:HC].rearrange("p t (c h) -> p t c h",
                                                   h=HEADS),
                    in1=att.unsqueeze(2).to_broadcast([P, T, OUT_C, HEADS]),
                    op=mybir.AluOpType.mult)

                ops = ppool.tile([P, GROW], F32, space="PSUM", tag="ops")
                for t in range(T):
                    nc.tensor.matmul(out=ops[:], lhsT=ident_sb[:],
                                     rhs=gath[:, t, :],
                                     start=(t == 0), stop=(t == T - 1))

                # normalize + head mean + bias
                dr = spool.tile([P, HEADS], F32, tag="dr")
                nc.vector.tensor_scalar(out=dr[:], in0=ops[:, HC:GROW],
                                        scalar1=float(HEADS),
                                        scalar2=DENOM_EPS,
                                        op0=mybir.AluOpType.mult,
                                        op1=mybir.AluOpType.add)
                rcp = spool.tile([P, HEADS], F32, tag="rcp")
                nc.vector.reciprocal(out=rcp[:], in_=dr[:])
                on = opool.tile([P, OUT_C, HEADS], F16, tag="on")
                nc.vector.tensor_tensor(
                    out=on[:],
                    in0=ops[:, 0:HC].rearrange("p (c h) -> p c h", h=HEADS),
                    in1=rcp[:].unsqueeze(1).to_broadcast([P, OUT_C, HEADS]),
                    op=mybir.AluOpType.mult)
                t1 = spool.tile([P, OUT_C, 4], F16, tag="t1")
                nc.vector.tensor_tensor(out=t1[:], in0=on[:, :, 0:4],
                                        in1=on[:, :, 4:8],
                                        op=mybir.AluOpType.add)
                t2 = spool.tile([P, OUT_C, 2], F16, tag="t2")
                nc.vector.tensor_tensor(out=t2[:], in0=t1[:, :, 0:2],
                                        in1=t1[:, :, 2:4],
                                        op=mybir.AluOpType.add)
                t3 = spool.tile([P, OUT_C], F16, tag="t3")
                nc.vector.tensor_tensor(out=t3[:],
                                        in0=t2[:, :, 0:1].rearrange(
                                            "p c one -> p (c one)"),
                                        in1=t2[:, :, 1:2].rearrange(
                                            "p c one -> p (c one)"),
                                        op=mybir.AluOpType.add)
                ob = opool.tile([P, OUT_C], F32, tag="ob")
                nc.vector.tensor_tensor(out=ob[:], in0=t3[:], in1=bias_sb[:],
                                        op=mybir.AluOpType.add)
                nc.sync.dma_start(out=out_d[g * P:(g + 1) * P, :], in_=ob[:])
                off += T

    return nc


# ----------------------------------------------------------------------------
# Walrus in this container accepts at most ONE semaphore wait per engine
# instruction.  Rebuild blocks, hoisting extra waits onto NOP carriers
# placed immediately before the instruction (same engine) — semantically
# identical (the engine just stalls one instruction earlier).
# ----------------------------------------------------------------------------

def _engine_obj(nc, engine):
    return {
        mb.EngineType.PE: nc.tensor,
        mb.EngineType.DVE: nc.vector,
        mb.EngineType.Activation: nc.scalar,
        mb.EngineType.SP: nc.sync,
        mb.EngineType.Pool: nc.gpsimd,
    }[engine]


def legalize_waits(nc, max_waits=1):
    Op = nc.isa.Opcode
    for f in nc.m.functions:
        new_blocks = []
        for blk in f.blocks:
            out = []
            for inst in blk.instructions:
                si = inst.sync_info
                waits = list(si.on_wait) if si is not None else []
                if len(waits) > max_waits:
                    eng = _engine_obj(nc, inst.engine)
                    extra, keep = waits[:-max_waits], waits[-max_waits:]
                    opc = (Op.NEURON_ISA_TPB_OPCODE_ENGINE_NOP
                           if inst.engine == mb.EngineType.Pool
                           else Op.NEURON_ISA_TPB_OPCODE_NOP)
                    for w in extra:
                        nop = eng._isa(opc, {})
                        nop.sync_info = mb.SyncInfo(on_wait=[w], on_update=[])
                        out.append(nop)
                    inst.sync_info = mb.SyncInfo(on_wait=keep,
                                                 on_update=list(si.on_update))
                out.append(inst)
            new_blocks.append(mb.BasicBlock(
                name=blk.name, instructions=out,
                IsPredicated=blk.IsPredicated, IsExit=blk.IsExit,
                IsLoopEntry=blk.IsLoopEntry))
        f.blocks = new_blocks
    return nc


# ----------------------------------------------------------------------------
# Full kernel: host prep -> run on 8 cores -> unshard
# ----------------------------------------------------------------------------

def make_in_maps(x, plan, consts, n_cores=N_CORES):
    x16 = np.asarray(x, dtype=np.float16)
    n = x16.shape[0]
    NROWS = plan["NROWS"]
    in_maps = []
    for c in range(n_cores):
        perm = plan["perms"][c]
        xp = np.zeros((NROWS, P), dtype=np.float16)
        rows = np.where(perm >= 0)[0]
        xp[rows] = x16[perm[rows]]
        m = dict(xt=np.ascontiguousarray(xp.T),
                 wcat=consts["wcat"], padrow=consts["padrow"],
                 ident=consts["ident"], bias_rep=consts["bias_rep"],
                 gidx=plan["gidx"][c])
        in_maps.append(m)
    return in_maps


def unshard(results, plan, n_nodes):
    out = np.zeros((n_nodes, OUT_C), dtype=np.float32)
    for c in range(N_CORES):
        no = plan["node_of"][c]
        res = results[c]["out"]
        mask = no >= 0
        out[no[mask]] = res[mask]
    return out


_CACHE = {}


def kernel(x, edge_index, batch, W, att_src, att_dst, bias):
    x = np.ascontiguousarray(np.asarray(x, dtype=np.float32))
    n_nodes = x.shape[0]
    plan = plan_dst(np.asarray(edge_index), n_nodes)
    key = (n_nodes, plan["G"], tuple(plan["T_slot"]), plan["NROWS"])
    if key not in _CACHE:
        nc = build_bass(plan["G"], plan["T_slot"], plan["sumT"],
                        plan["NROWS"])
        legalize_waits(nc)
        _CACHE[key] = nc
    nc = _CACHE[key]
    consts = host_constants(W, att_src, att_dst, bias)
    in_maps = make_in_maps(x, plan, consts)
    from concourse.bass_utils import run_bass_kernel_spmd
    res = run_bass_kernel_spmd(nc, in_maps, list(range(N_CORES)), trace=False)
    return unshard(res.results, plan, n_nodes)


# revision 14
# speedup vs baseline: 3.6231x; 1.1412x over previous
"""Self-contained GAT (PyG GATConv, concat=False) Bass/Tile kernel for 8
Trainium2 NeuronCores.  kernel(**inputs) takes the full-graph inputs and
returns the full [N, 32] output.

Strategy (dst-partition): nodes are sorted by in-degree (self-loops included)
and packed into groups of 128; inside a group the PARTITION index is the
destination node, and the free dim holds that node's incoming edges
(T = group max degree slots per node).  This removes the one-hot scatter
matrices entirely: segment-sum over a group's edges is T identity-matmuls
accumulated in PSUM.  Groups are snake-dealt to the 8 cores; all cores run
one shared program (per-group T schedule identical), per-core differences
live in the data (a per-core node permutation makes each core's own groups
occupy rows [g*128,(g+1)*128) of its private `ha` scratch).

Per core:
  Phase A: ha[r] = [h_cmajor(256) | a_src(8) | a_dst(8)] fp16 for ALL rows
           (x pre-transposed+fp16 on host, one 272-col fp16 matmul per
           128-row tile; PSUM->SBUF cast copies split ACT/DVE).
  Phase B per group: one multi-row indirect DMA gathers all T*128 edge
           source rows (264 cols); a_dst of the group's own nodes comes from
           a strided slice load; attention = exp(leaky(a_src+a_dst) - 8)
           computed in-place (DVE leaky, ACT exp, shift avoids fp16
           overflow; softmax is shift-invariant so no max pass is needed);
           messages multiply in fp16 2x mode (head-minor layout keeps the
           broadcast packed); T identity-matmuls accumulate [128,264] in
           PSUM (256 msg cols + 8 denominator cols); normalize + head-mean
           + bias on DVE; store [128,32].
Empty edge slots gather a pad row with a_src=-60000 so their exp underflows
to exactly 0 in fp16.
"""

import math

import numpy as np

import concourse.tile as tile
import concourse.mybir as mb
from concourse import bass, mybir

P = 128
F32 = mybir.dt.float32
F16 = mybir.dt.float16
I32 = mybir.dt.int32

HEADS = 8
OUT_C = 32
HC = HEADS * OUT_C          # 256
ROW = HC + 2 * HEADS        # 272 per ha row: h(c-major) | a_src | a_dst
GROW = HC + HEADS           # 264 gathered cols per edge row
NEG_SLOPE = 0.2
EXP_SHIFT = -8.0            # exp(s - 8): keeps fp16 in range; shift-invariant
DENOM_EPS = 1e-30
PAD_ASRC = -60000.0         # pad-slot a_src: exp underflows to exact 0
N_CORES = 8
CHUNK = 4                   # phase-A node tiles per DMA chunk


# ----------------------------------------------------------------------------
# Host-side planning
# ----------------------------------------------------------------------------

def plan_dst(edge_index: np.ndarray, n_nodes: int, n_cores: int = N_CORES):
    src = np.asarray(edge_index[0], dtype=np.int64)
    dst = np.asarray(edge_index[1], dtype=np.int64)
    loops = np.arange(n_nodes, dtype=np.int64)
    src = np.concatenate([src, loops])
    dst = np.concatenate([dst, loops])

    deg = np.bincount(dst, minlength=n_nodes).astype(np.int64)   # >= 1
    node_order = np.argsort(-deg, kind="stable")                 # desc degree
    node_rank = np.empty(n_nodes, dtype=np.int64)
    node_rank[node_order] = np.arange(n_nodes)

    n_groups = math.ceil(n_nodes / P)
    n_groups = math.ceil(n_groups / n_cores) * n_cores           # e.g. 392
    G = n_groups // n_cores
    NROWS = n_groups * P
    PADROW = NROWS

    # edges sorted by dst; per-dst CSR and within-dst counter
    order_e = np.argsort(dst, kind="stable")
    src_s = src[order_e]
    dst_s = dst[order_e]
    csr = np.zeros(n_nodes + 1, dtype=np.int64)
    np.cumsum(deg, out=csr[1:])
    t_of_edge = np.arange(len(dst_s)) - csr[dst_s]

    # per-rank-group T; slot j serves ranks [8j, 8j+8) => T_slot = Tg[8j]
    Tg = np.ones(n_groups, dtype=np.int64)
    for k in range(n_groups):
        if k * P < n_nodes:
            Tg[k] = deg[node_order[k * P]]
    T_slot = Tg[n_cores * np.arange(G)].astype(np.int64)
    offs = np.zeros(G + 1, dtype=np.int64)
    np.cumsum(T_slot, out=offs[1:])
    sumT = int(offs[-1])

    # snake deal: slot j, core c -> rank 8j + (c if j even else 7-c)
    rank_core = np.empty(n_groups, dtype=np.int64)
    rank_slot = np.empty(n_groups, dtype=np.int64)
    for j in range(G):
        for c in range(n_cores):
            r = n_cores * j + (c if j % 2 == 0 else n_cores - 1 - c)
            rank_core[r] = c
            rank_slot[r] = j

    # per-node placement
    n_core = rank_core[node_rank // P]          # owning core of each node
    n_slot = rank_slot[node_rank // P]          # group slot on that core
    n_part = node_rank % P                      # partition within group

    # per-core node permutation: own groups first (slot-major), then the rest
    perms = []       # perms[c][row] = node id or -1
    pos = []         # pos[c][node] = row of node in core c's ha
    node_of = []     # node_of[c][G*P] = node id or -1 (for unshard)
    for c in range(n_cores):
        perm = np.full(NROWS, -1, dtype=np.int64)
        own = n_core == c
        own_rows = n_slot[own] * P + n_part[own]
        own_nodes = np.where(own)[0]
        perm[own_rows] = own_nodes
        rest = np.where(~own)[0]
        perm[G * P:G * P + len(rest)] = rest
        p = np.empty(n_nodes, dtype=np.int64)
        rows = np.where(perm >= 0)[0]
        p[perm[rows]] = rows
        perms.append(perm)
        pos.append(p)
        node_of.append(perm[:G * P].copy())

    # per-core gather index maps
    gidx = np.full((n_cores, P, sumT), PADROW, dtype=np.int32)
    e_core = n_core[dst_s]
    e_col = offs[n_slot[dst_s]] + t_of_edge
    e_part = n_part[dst_s]
    for c in range(n_cores):
        m = e_core == c
        gidx[c, e_part[m], e_col[m]] = pos[c][src_s[m]].astype(np.int32)

    return dict(G=G, T_slot=T_slot.tolist(), offs=offs, sumT=sumT,
                NROWS=NROWS, PADROW=PADROW, perms=perms, node_of=node_of,
                gidx=gidx)


def host_constants(W, att_src, att_dst, bias):
    W = np.asarray(W, dtype=np.float32)          # [128, 256]
    att_src = np.asarray(att_src, dtype=np.float32)
    att_dst = np.asarray(att_dst, dtype=np.float32)
    Wr = W.reshape(P, HEADS, OUT_C)
    wcat = np.zeros((P, ROW), dtype=np.float32)
    wcat[:, :HC] = Wr.transpose(0, 2, 1).reshape(P, HC)       # col = c*8+h
    wcat[:, HC:HC + HEADS] = np.einsum("khc,hc->kh", Wr, att_src)
    wcat[:, HC + HEADS:] = np.einsum("khc,hc->kh", Wr, att_dst)
    padrow = np.zeros((1, ROW), dtype=np.float16)
    padrow[0, HC:HC + HEADS] = PAD_ASRC
    ident = np.eye(P, dtype=np.float16)
    bias_rep = np.tile(np.asarray(bias, dtype=np.float32)[None, :], (P, 1))
    return dict(wcat=wcat.astype(np.float16), padrow=padrow, ident=ident,
                bias_rep=bias_rep)


# ----------------------------------------------------------------------------
# Bass program (shared by all cores)
# ----------------------------------------------------------------------------

def build_bass(G: int, T_slot, sumT: int, NROWS: int):
    nc = bass.Bass(trn_type="TRN2", dynamic_dma_scratch_size=65536)

    xt_d = nc.dram_tensor("xt", [P, NROWS], F16, kind="ExternalInput")
    wcat_d = nc.dram_tensor("wcat", [P, ROW], F16, kind="ExternalInput")
    pad_d = nc.dram_tensor("padrow", [1, ROW], F16, kind="ExternalInput")
    ident_d = nc.dram_tensor("ident", [P, P], F16, kind="ExternalInput")
    bias_d = nc.dram_tensor("bias_rep", [P, OUT_C], F32, kind="ExternalInput")
    gidx_d = nc.dram_tensor("gidx", [P, sumT], I32, kind="ExternalInput")
    out_d = nc.dram_tensor("out", [G * P, OUT_C], F32, kind="ExternalOutput")

    ha_d = nc.dram_tensor("ha", [NROWS + 1, ROW], F16)   # +1 pad row

    n_tiles = NROWS // P
    n_chunks = n_tiles // CHUNK

    # ---------------- Phase A: ha = [x@W | x@WAs | x@WAd] (fp16) ------------
    with tile.TileContext(nc) as tc:
        with (
            tc.tile_pool(name="aconst", bufs=1) as cpool,
            tc.tile_pool(name="asb", bufs=3) as spool,
            tc.tile_pool(name="aps", bufs=2, space="PSUM") as ppool,
        ):
            wcat_sb = cpool.tile([P, ROW], F16, name="wcat_sb")
            nc.sync.dma_start(out=wcat_sb[:], in_=wcat_d[:, :])
            pad_sb = cpool.tile([1, ROW], F16, name="pad_sb")
            nc.sync.dma_start(out=pad_sb[:], in_=pad_d[:, :])
            nc.sync.dma_start(out=ha_d[NROWS:NROWS + 1, :], in_=pad_sb[:])

            for i in range(n_chunks):
                c0 = i * CHUNK * P
                xt = spool.tile([P, CHUNK * P], F16, tag="xt")
                nc.sync.dma_start(out=xt[:], in_=xt_d[:, c0:c0 + CHUNK * P])
                stage = spool.tile([P, CHUNK, ROW], F16, tag="stage")
                # bank-aligned PSUM (512 f32 = 1 bank per tile) so one wide
                # strided copy drains all CHUNK matmul outputs
                hps = ppool.tile([P, CHUNK, 512], F32, space="PSUM",
                                 tag="hps")
                for j in range(CHUNK):
                    nc.tensor.matmul(out=hps[:, j, 0:ROW],
                                     lhsT=xt[:, j * P:(j + 1) * P],
                                     rhs=wcat_sb[:], start=True, stop=True)
                if i % 2 == 0:
                    nc.vector.tensor_copy(out=stage[:], in_=hps[:, :, 0:ROW])
                else:
                    nc.scalar.activation(
                        out=stage[:], in_=hps[:, :, 0:ROW],
                        func=mybir.ActivationFunctionType.Copy)
                eng = nc.sync if i % 2 == 0 else nc.scalar
                eng.dma_start(
                    out=ha_d[c0:c0 + CHUNK * P, :].rearrange(
                        "(j p) c -> p j c", p=P),
                    in_=stage[:],
                )

    # ---------------- Phase B: per-group edge aggregation -------------------
    with tile.TileContext(nc) as tc:
        with (
            tc.tile_pool(name="bconst", bufs=1) as cpool,
            tc.tile_pool(name="bgath", bufs=3) as gpool,
            tc.tile_pool(name="bsmall", bufs=4) as spool,
            tc.tile_pool(name="bout", bufs=3) as opool,
            tc.tile_pool(name="bps", bufs=4, space="PSUM") as ppool,
        ):
            ident_sb = cpool.tile([P, P], F16, name="ident_sb")
            nc.sync.dma_start(out=ident_sb[:], in_=ident_d[:, :])
            bias_sb = cpool.tile([P, OUT_C], F32, name="bias_sb")
            nc.sync.dma_start(out=bias_sb[:], in_=bias_d[:, :])
            gidx_sb = cpool.tile([P, sumT], I32, name="gidx_sb")
            nc.sync.dma_start(out=gidx_sb[:], in_=gidx_d[:, :])
            shift_sb = cpool.tile([P, 1], F32, name="shift_sb")
            nc.vector.memset(shift_sb[:], EXP_SHIFT)


            off = 0
            for g in range(G):
                T = int(T_slot[g])
                gath = gpool.tile([P, T, GROW], F16, tag="gath")
                nc.gpsimd.indirect_dma_start(
                    out=gath[:, :, :],
                    out_offset=None,
                    in_=ha_d[:, :],
                    in_offset=bass.IndirectOffsetOnAxis(
                        ap=gidx_sb[:, off:off + T], axis=0),
                )
                adst = spool.tile([P, HEADS], F16, tag="adst")
                nc.sync.dma_start(out=adst[:],
                                  in_=ha_d[g * P:(g + 1) * P, HC + HEADS:ROW])

                att = gath[:, :, HC:GROW]
                # s = a_src + a_dst ; leaky = max(s, 0.2*s) ; ex = exp(leaky-8)
                nc.vector.tensor_tensor(
                    out=att, in0=att,
                    in1=adst[:].unsqueeze(1).to_broadcast([P, T, HEADS]),
                    op=mybir.AluOpType.add)
                sl = spool.tile([P, T, HEADS], F16, tag="sl")
                nc.scalar.activation(out=sl[:], in_=att,
                                     func=mybir.ActivationFunctionType.Copy,
                                     scale=NEG_SLOPE)
                nc.vector.tensor_tensor(out=att, in0=att, in1=sl[:],
                                        op=mybir.AluOpType.max)
                nc.scalar.activation(out=att, in_=att,
                                     func=mybir.ActivationFunctionType.Exp,
                                     bias=shift_sb[:])
                # msg = h * alpha_unnorm (head-minor keeps broadcast packed)
                nc.vector.tensor_tensor(
                    out=gath[:, :, 0:HC].rearrange("p t (c h) -> p t c h",
                                                   h=HEADS),
                    in0=gath[:, :, 0:HC].rearrange("p t (c h) -> p t c h",
                                                   h=HEADS),
                    in1=att.unsqueeze(2).to_broadcast([P, T, OUT_C, HEADS]),
                    op=mybir.AluOpType.mult)

                ops = ppool.tile([P, GROW], F32, space="PSUM", tag="ops")
                for t in range(T):
                    nc.tensor.matmul(out=ops[:], lhsT=ident_sb[:],
                                     rhs=gath[:, t, :],
                                     start=(t == 0), stop=(t == T - 1))

                # normalize + head mean + bias
                dr = spool.tile([P, HEADS], F32, tag="dr")
                nc.scalar.activation(out=dr[:], in_=ops[:, HC:GROW],
                                     func=mybir.ActivationFunctionType.Copy,
                                     scale=float(HEADS), bias=DENOM_EPS)
                rcp = spool.tile([P, HEADS], F32, tag="rcp")
                nc.vector.reciprocal(out=rcp[:], in_=dr[:])
                on = opool.tile([P, OUT_C, HEADS], F16, tag="on")
                nc.vector.tensor_tensor(
                    out=on[:],
                    in0=ops[:, 0:HC].rearrange("p (c h) -> p c h", h=HEADS),
                    in1=rcp[:].unsqueeze(1).to_broadcast([P, OUT_C, HEADS]),
                    op=mybir.AluOpType.mult)
                t1 = spool.tile([P, OUT_C, 4], F16, tag="t1")
                nc.vector.tensor_tensor(out=t1[:], in0=on[:, :, 0:4],
                                        in1=on[:, :, 4:8],
                                        op=mybir.AluOpType.add)
                t2 = spool.tile([P, OUT_C, 2], F16, tag="t2")
                nc.vector.tensor_tensor(out=t2[:], in0=t1[:, :, 0:2],
                                        in1=t1[:, :, 2:4],
                                        op=mybir.AluOpType.add)
                t3 = spool.tile([P, OUT_C], F16, tag="t3")
                nc.vector.tensor_tensor(out=t3[:],
                                        in0=t2[:, :, 0:1].rearrange(
                                            "p c one -> p (c one)"),
                                        in1=t2[:, :, 1:2].rearrange(
                                            "p c one -> p (c one)"),
                                        op=mybir.AluOpType.add)
                ob = opool.tile([P, OUT_C], F32, tag="ob")
                nc.vector.tensor_tensor(out=ob[:], in0=t3[:], in1=bias_sb[:],
                                        op=mybir.AluOpType.add)
                nc.sync.dma_start(out=out_d[g * P:(g + 1) * P, :], in_=ob[:])
                off += T

    return nc


# ----------------------------------------------------------------------------
# Walrus in this container accepts at most ONE semaphore wait per engine
# instruction.  Rebuild blocks, hoisting extra waits onto NOP carriers
# placed immediately before the instruction (same engine) — semantically
# identical (the engine just stalls one instruction earlier).
# ----------------------------------------------------------------------------

def _engine_obj(nc, engine):
    return {
        mb.EngineType.PE: nc.tensor,
        mb.EngineType.DVE: nc.vector,
        mb.EngineType.Activation: nc.scalar,
        mb.EngineType.SP: nc.sync,
        mb.EngineType.Pool: nc.gpsimd,
    }[engine]


def legalize_waits(nc, max_waits=1):
    Op = nc.isa.Opcode
    for f in nc.m.functions:
        new_blocks = []
        for blk in f.blocks:
            out = []
            for inst in blk.instructions:
                si = inst.sync_info
                waits = list(si.on_wait) if si is not None else []
                if len(waits) > max_waits:
                    eng = _engine_obj(nc, inst.engine)
                    extra, keep = waits[:-max_waits], waits[-max_waits:]
                    opc = (Op.NEURON_ISA_TPB_OPCODE_ENGINE_NOP
                           if inst.engine == mb.EngineType.Pool
                           else Op.NEURON_ISA_TPB_OPCODE_NOP)
                    for w in extra:
                        nop = eng._isa(opc, {})
                        nop.sync_info = mb.SyncInfo(on_wait=[w], on_update=[])
                        out.append(nop)
                    inst.sync_info = mb.SyncInfo(on_wait=keep,
                                                 on_update=list(si.on_update))
                out.append(inst)
            new_blocks.append(mb.BasicBlock(
                name=blk.name, instructions=out,
                IsPredicated=blk.IsPredicated, IsExit=blk.IsExit,
                IsLoopEntry=blk.IsLoopEntry))
        f.blocks = new_blocks
    return nc


# ----------------------------------------------------------------------------
# Full kernel: host prep -> run on 8 cores -> unshard
# ----------------------------------------------------------------------------

def make_in_maps(x, plan, consts, n_cores=N_CORES):
    x16 = np.asarray(x, dtype=np.float16)
    n = x16.shape[0]
    NROWS = plan["NROWS"]
    in_maps = []
    for c in range(n_cores):
        perm = plan["perms"][c]
        xp = np.zeros((NROWS, P), dtype=np.float16)
        rows = np.where(perm >= 0)[0]
        xp[rows] = x16[perm[rows]]
        m = dict(xt=np.ascontiguousarray(xp.T),
                 wcat=consts["wcat"], padrow=consts["padrow"],
                 ident=consts["ident"], bias_rep=consts["bias_rep"],
                 gidx=plan["gidx"][c])
        in_maps.append(m)
    return in_maps


def unshard(results, plan, n_nodes):
    out = np.zeros((n_nodes, OUT_C), dtype=np.float32)
    for c in range(N_CORES):
        no = plan["node_of"][c]
        res = results[c]["out"]
        mask = no >= 0
        out[no[mask]] = res[mask]
    return out


_CACHE = {}


def kernel(x, edge_index, batch, W, att_src, att_dst, bias):
    x = np.ascontiguousarray(np.asarray(x, dtype=np.float32))
    n_nodes = x.shape[0]
    plan = plan_dst(np.asarray(edge_index), n_nodes)
    key = (n_nodes, plan["G"], tuple(plan["T_slot"]), plan["NROWS"])
    if key not in _CACHE:
        nc = build_bass(plan["G"], plan["T_slot"], plan["sumT"],
                        plan["NROWS"])
        legalize_waits(nc)
        _CACHE[key] = nc
    nc = _CACHE[key]
    consts = host_constants(W, att_src, att_dst, bias)
    in_maps = make_in_maps(x, plan, consts)
    from concourse.bass_utils import run_bass_kernel_spmd
    res = run_bass_kernel_spmd(nc, in_maps, list(range(N_CORES)), trace=False)
    return unshard(res.results, plan, n_nodes)
